# revision 27
# baseline (speedup 1.0000x reference)
"""Trainium2 Bass kernel for masked multi-head cross-attention.

Reference computation (shapes):
  q       (32, 1024, 512)  f32
  K, V    (8, 32, 1024, 64) f32
  mask    (32, 1024, 1024) bool   True = masked out
  W_query (8, 512, 64), W_out (8, 64, 512)
  out     (32, 1024, 512)  f32

  Q = einsum('bqi,hik->hbqk', q, W_query); S = Q K^T / sqrt(64)
  A = softmax(S masked with -inf), zeroed at masked; out = sum_h (A V) W_out

Strategy: data-parallel over batch, 4 batches per NeuronCore on 8 cores.
On-chip layout keeps scores transposed (S.T: g on partitions, q free) so
softmax denominators come from a ones-column appended to V and the exp()
output feeds the A@V matmul directly as the moving operand - no PE
transposes. Per-q normalization (1/denominator) is distributed across
partitions via a reshape DMA + reciprocal + broadcast DMA, applied during
the PSUM->SBUF move of O^T.
"""

import math

import numpy as np

N_HEADS, INPUT_DIM, EMBED_DIM, KEY_DIM = 8, 512, 512, 64
BATCH, N_QUERY, GRAPH = 32, 1024, 1024
NORM = 1.0 / math.sqrt(KEY_DIM)
N_CORES = 8
B_PER_CORE = BATCH // N_CORES

# compute dtype for matmul operands ("float32" exact ~1.4ms; "bfloat16"
# ~0.32ms at rel err ~6e-3 vs the f32 reference)
COMPUTE_DT = "bfloat16"
# fraction of mask-multiplies routed to GPSIMD instead of DVE (0, 1, or 2=alt)
MASK_ON_GPSIMD = 0


class Cfg:
    def __init__(self, nb=B_PER_CORE, h=N_HEADS, g=GRAPH, q=N_QUERY,
                 i=INPUT_DIM, e=EMBED_DIM, kd=KEY_DIM, dt=None):
        dt = dt or COMPUTE_DT
        assert h % 2 == 0 and g % 128 == 0 and q % 128 == 0 and i % 128 == 0
        assert kd == 64
        self.nb, self.h, self.g, self.q, self.i, self.e, self.kd = nb, h, g, q, i, e, kd
        self.np_ = h // 2          # head pairs
        self.gc = g // 128         # g chunks of 128
        self.ic = i // 128         # input-dim chunks of 128
        self.qt = q // 128         # q tiles of 128
        self.t = q // 128          # free elems per partition in reshaped denom
        self.nqc = (q + 511) // 512  # 512-wide N chunks over q
        self.dt_name = dt


import os

APOOL_BUFS = int(os.environ.get("APOOL_BUFS", "4"))


def make_pools(ctx, tc, cfg):
    H = cfg.h
    return {
        "const": ctx.enter_context(tc.tile_pool(name="const", bufs=1)),
        "inp": ctx.enter_context(tc.tile_pool(name="inp", bufs=2)),
        "qt2p": ctx.enter_context(tc.tile_pool(name="qt2p", bufs=2)),
        "apool": ctx.enter_context(tc.tile_pool(name="apool", bufs=APOOL_BUFS)),
        "avsbp": ctx.enter_context(tc.tile_pool(name="avsbp", bufs=3)),
        "otp": ctx.enter_context(tc.tile_pool(name="otp", bufs=H + 1)),
        "smalls": ctx.enter_context(tc.tile_pool(name="smalls", bufs=3)),
        "obp": ctx.enter_context(tc.tile_pool(name="obp", bufs=3)),
        "dramp": ctx.enter_context(tc.tile_pool(name="dramp", bufs=3, space="DRAM")),
        # scores tiles [128, Q] f32 = 2 banks; big: f32 qproj/av/outproj
        "scp": ctx.enter_context(tc.tile_pool(name="scp", bufs=2, space="PSUM")),
        "psum": ctx.enter_context(tc.tile_pool(name="psum", bufs=2, space="PSUM")),
    }


def build_body(ctx, tc, outs, ins, cfg, pools=None):
    """Emit the kernel IR. ins/outs are dicts of DRAM APs."""
    import concourse.mybir as mybir

    nc = tc.nc
    f32 = mybir.dt.float32
    bf16 = mybir.dt.bfloat16
    dt = getattr(mybir.dt, cfg.dt_name)
    EXP = mybir.ActivationFunctionType.Exp

    qt_d, kt_d, va_d, keep_d = ins["qt"], ins["kt"], ins["va"], ins["keep"]
    wq_d, wo_d = ins["wq"], ins["wo"]
    out_d = outs["out"]

    NB, H, G, Q, E = cfg.nb, cfg.h, cfg.g, cfg.q, cfg.e
    NP, GC, IC, QT, T = cfg.np_, cfg.gc, cfg.ic, cfg.qt, cfg.t

    def nq_slices():
        for s in range(0, Q, 512):
            yield slice(s, min(s + 512, Q))

    if pools is None:
        pools = make_pools(ctx, tc, cfg)
    const, inp, qt2p, apool, avsbp, otp, smalls, obp, dramp, scp, psum = (
        pools["const"], pools["inp"], pools["qt2p"], pools["apool"],
        pools["avsbp"], pools["otp"], pools["smalls"], pools["obp"],
        pools["dramp"], pools["scp"], pools["psum"])

    # weights, loaded once
    wq_sb = const.tile([128, NP, IC, 128], dt, tag="wq_sb")
    nc.sync.dma_start(out=wq_sb, in_=wq_d)
    wo_sb = const.tile([64, H, E], dt, tag="wo_sb")
    nc.sync.dma_start(out=wo_sb, in_=wo_d)

    for b in range(NB):
        qt_b = inp.tile([128, IC, Q], dt, tag="qt_b")
        nc.sync.dma_start(out=qt_b, in_=qt_d[b])
        kt_b = inp.tile([128, NP, G], dt, tag="kt_b")
        nc.sync.dma_start(out=kt_b, in_=kt_d[b])
        va_b = inp.tile([128, H, GC, 65], dt, tag="va_b")
        nc.sync.dma_start(out=va_b, in_=va_d[b])
        keep_b = inp.tile([128, GC, Q], bf16, tag="keep_b")
        nc.sync.dma_start(out=keep_b, in_=keep_d[b])

        ot_tiles = []
        for pr in range(NP):
            # --- Q projection for head pair pr: QT2[128(kd of 2 heads), Q]
            qp_ps = psum.tile([128, Q], f32, tag="ps")
            for ic in range(IC):
                for nsl in nq_slices():
                    nc.tensor.matmul(
                        qp_ps[:, nsl],
                        lhsT=wq_sb[:, pr, ic, :],
                        rhs=qt_b[:, ic, nsl],
                        start=(ic == 0),
                        stop=(ic == IC - 1),
                    )
            qt2 = qt2p.tile([128, Q], dt, tag="qt2")
            nc.vector.tensor_copy(qt2, qp_ps)

            # --- scores + exp + mask + A@V for both heads of the pair,
            #     interleaved over g chunks (one joint exp per chunk)
            av_pair = [psum.tile([65, Q], f32, tag="ps", name=f"av{hi}")
                       for hi in range(2)]
            for gc2 in range(GC):
                for hi in range(2):
                    psl = slice(64 * hi, 64 * hi + 64)
                    sc = scp.tile([128, Q], f32, tag="sc")
                    for nsl in nq_slices():
                        nc.tensor.matmul(
                            sc[:, nsl],
                            lhsT=kt_b[psl, pr, 128 * gc2:128 * gc2 + 128],
                            rhs=qt2[psl, nsl],
                            start=True,
                            stop=True,
                        )
                    # exp (PSUM f32 -> SBUF bf16), then zero masked entries
                    aexp = apool.tile([128, Q], dt, tag="aexp")
                    nc.scalar.activation(out=aexp, in_=sc, func=EXP)
                    if MASK_ON_GPSIMD == 0:
                        eng = nc.vector
                    elif MASK_ON_GPSIMD == 1:
                        eng = nc.gpsimd
                    else:
                        eng = nc.vector if (gc2 + hi) % MASK_ON_GPSIMD else nc.gpsimd
                    eng.tensor_mul(aexp, aexp, keep_b[:, gc2, :])
                    for nsl in nq_slices():
                        nc.tensor.matmul(
                            av_pair[hi][:, nsl],
                            lhsT=va_b[:, 2 * pr + hi, gc2, :],
                            rhs=aexp[:, nsl],
                            start=(gc2 == 0),
                            stop=(gc2 == GC - 1),
                        )
            # --- softmax normalization per head:
            #     av rows 0..63 = O^T (unnormalized), row 64 = denominator
            for hi in range(2):
                av_ps = av_pair[hi]
                import concourse.bass as bass
                # move O_aug^T off PSUM (frees the accumulation slot early)
                av_sb = avsbp.tile([65, Q], f32, tag="av_sb")
                nc.vector.tensor_copy(av_sb, av_ps)
                d2 = smalls.tile([128, T], f32, tag="d2")
                src = av_sb[64:65, :].rearrange("o (p t) -> o p t", p=128)
                nc.gpsimd.dma_start(out=d2, in_=src)
                r2 = smalls.tile([128, T], f32, tag="r2")
                nc.vector.reciprocal(r2, d2)
                # broadcast r2 across 64 partitions: rbc[dp, p*T+t] = r2[p, t]
                # (bounce via DRAM: SBUF source APs cannot have step-0
                #  partition dims, DRAM sources can)
                rd = dramp.tile([128, T], f32, tag="rd")
                nc.sync.dma_start(out=rd, in_=r2)
                rbc = smalls.tile([64, Q], f32, tag="rbc")
                rdflat = rd[:, :].rearrange("p t -> (p t)")
                bsrc = bass.AP(tensor=rdflat.tensor, offset=rdflat.offset,
                               ap=[[0, 64]] + list(rdflat.ap))
                nc.gpsimd.dma_start(out=rbc, in_=bsrc)
                ot = otp.tile([64, Q], dt, tag="ot")
                nc.vector.tensor_mul(ot, av_sb[0:64, :], rbc)
                ot_tiles.append(ot)

        # --- output projection: out[q,e] = sum_h O_h[q,:] @ W_out[h]
        for qi in range(QT):
            op_ps = psum.tile([128, E], f32, tag="ps")
            for h in range(H):
                nc.tensor.matmul(
                    op_ps,
                    lhsT=ot_tiles[h][:, 128 * qi:128 * qi + 128],
                    rhs=wo_sb[:, h, :],
                    start=(h == 0),
                    stop=(h == H - 1),
                )
            ob = obp.tile([128, E], f32, tag="ob")
            nc.vector.tensor_copy(ob, op_ps)
            nc.sync.dma_start(out=out_d[b, 128 * qi:128 * qi + 128, :], in_=ob)


def build_nc(cfg, reps=1):
    """Build + compile the Bacc module with DRAM tensors named per prep_core.

    reps>1 emits the body multiple times (identical work) for timing via
    the (t_N - t_1)/(N-1) delta method."""
    from contextlib import ExitStack

    import concourse.mybir as mybir
    import concourse.tile as tile
    from concourse import bacc

    dt = getattr(mybir.dt, cfg.dt_name)
    f32 = mybir.dt.float32
    bf16 = mybir.dt.bfloat16

    nc = bacc.Bacc("TRN2", target_bir_lowering=False, debug=False)
    NB, H, G, Q, E = cfg.nb, cfg.h, cfg.g, cfg.q, cfg.e
    NP, GC, IC = cfg.np_, cfg.gc, cfg.ic

    ins = {
        "qt": nc.dram_tensor("qt", [NB, 128, IC, Q], dt, kind="ExternalInput").ap(),
        "kt": nc.dram_tensor("kt", [NB, 128, NP, G], dt, kind="ExternalInput").ap(),
        "va": nc.dram_tensor("va", [NB, 128, H, GC, 65], dt, kind="ExternalInput").ap(),
        "keep": nc.dram_tensor("keep", [NB, 128, GC, Q], bf16, kind="ExternalInput").ap(),
        "wq": nc.dram_tensor("wq", [128, NP, IC, 128], dt, kind="ExternalInput").ap(),
        "wo": nc.dram_tensor("wo", [64, H, E], dt, kind="ExternalInput").ap(),
    }
    outs = {
        "out": nc.dram_tensor("out", [NB, Q, E], f32, kind="ExternalOutput").ap(),
    }

    with tile.TileContext(nc) as tc:
        with ExitStack() as ctx:
            pools = make_pools(ctx, tc, cfg)
            for _ in range(reps):
                build_body(ctx, tc, outs, ins, cfg, pools=pools)
    nc.compile()
    return nc


def prep_core(q, K, V, mask, W_query, W_out, bsl, cfg):
    """Host-side shard prep for one core. bsl = slice of batches."""
    import ml_dtypes

    np_dt = np.float32 if cfg.dt_name == "float32" else ml_dtypes.bfloat16
    bf16 = ml_dtypes.bfloat16
    NB, H, G, Q, I, E, KD = cfg.nb, cfg.h, cfg.g, cfg.q, cfg.i, cfg.e, cfg.kd
    NP, GC, IC = cfg.np_, cfg.gc, cfg.ic

    qc = np.ascontiguousarray(q[bsl])          # [NB, Q, I]
    Kc = np.ascontiguousarray(K[:, bsl])       # [H, NB, G, KD]
    Vc = np.ascontiguousarray(V[:, bsl])       # [H, NB, G, KD]
    mc = np.ascontiguousarray(mask[bsl]).astype(bool)  # [NB, Q, G]

    # qt[b, p, ic, nq] = q[b, nq, 128*ic + p]
    qt = qc.transpose(0, 2, 1).reshape(NB, IC, 128, Q).transpose(0, 2, 1, 3)
    qt = np.ascontiguousarray(qt).astype(np_dt)

    # kt[b, sub*64+kd, pr, g] = K[2*pr+sub, b, g, kd]
    kt = Kc.transpose(1, 0, 3, 2).reshape(NB, NP, 2, KD, G)
    kt = kt.transpose(0, 2, 3, 1, 4).reshape(NB, 128, NP, G)
    kt = np.ascontiguousarray(kt).astype(np_dt)

    # va[b, p, h, gc, j] = V[h, b, 128*gc + p, j] (j<64), 1.0 at j=64
    va = np.concatenate([Vc, np.ones((H, NB, G, 1), np.float32)], axis=-1)
    va = va.transpose(1, 0, 2, 3).reshape(NB, H, GC, 128, 65).transpose(0, 3, 1, 2, 4)
    va = np.ascontiguousarray(va).astype(np_dt)

    # keep[b, p, gc, nq] = 1 - mask[b, nq, 128*gc + p]
    keep = (~mc).transpose(0, 2, 1).reshape(NB, GC, 128, Q).transpose(0, 2, 1, 3)
    keep = np.ascontiguousarray(keep).astype(bf16)

    # wq[p, pr, ic, m] = NORM * W_query[2*pr + (m>=64), 128*ic + p, m%64]
    wqs = (W_query * np.float32(NORM)).astype(np.float32)     # [H, I, KD]
    wq = wqs.reshape(NP, 2, IC, 128, KD).transpose(3, 0, 2, 1, 4)
    wq = wq.reshape(128, NP, IC, 128)
    wq = np.ascontiguousarray(wq).astype(np_dt)

    # wo[kd, h, e]
    wo = np.ascontiguousarray(W_out.transpose(1, 0, 2)).astype(np_dt)

    return {"qt": qt, "kt": kt, "va": va, "keep": keep, "wq": wq, "wo": wo}


_NC_CACHE = {}
TRACE = False
LAST_RESULT = None


def kernel(q, K, V, mask, W_query, W_out):
    global LAST_RESULT
    from concourse import bass_utils

    cfg = Cfg()
    key = (cfg.dt_name,)
    if key not in _NC_CACHE:
        _NC_CACHE[key] = build_nc(cfg)
    nc = _NC_CACHE[key]

    q = np.asarray(q)
    K = np.asarray(K)
    V = np.asarray(V)
    mask = np.asarray(mask)
    W_query = np.asarray(W_query)
    W_out = np.asarray(W_out)

    in_maps = []
    for c in range(N_CORES):
        bsl = slice(c * B_PER_CORE, (c + 1) * B_PER_CORE)
        in_maps.append(prep_core(q, K, V, mask, W_query, W_out, bsl, cfg))

    res = bass_utils.run_bass_kernel_spmd(
        nc, in_maps, core_ids=list(range(N_CORES)), trace=TRACE
    )
    LAST_RESULT = res
    out = np.empty((BATCH, N_QUERY, EMBED_DIM), np.float32)
    for c in range(N_CORES):
        out[c * B_PER_CORE:(c + 1) * B_PER_CORE] = res.results[c]["out"]
    return out
